# revision 47
# baseline (speedup 1.0000x reference)
"""Multi-head attention on 8 trn2 NeuronCores — TP-direct + key compaction.

Sharding: pure tensor-parallel for Q/K/V — each core reads the FULL
(bf16, host-transposed) query/key/value activations and projects only its
own 2 heads' 128 output dims (weight slices are tiny), so there are NO
collectives before attention.  The only collective is the AllToAll that
re-shards attention output from head-sharded to row-sharded before the
out-projection, split into two per-batch halves so the first overlaps
batch-1 attention compute.

Key compaction: the mask kills ~half the keys (V rows zeroed + keep-mask
denominator in the reference semantics).  Masked keys contribute exact
zeros, so the host gathers only unmasked key/value rows per batch (padded
to a multiple of 128, keep=0 on pads).  Scores / exp / attn@V shrink by
~2x with bitwise-identical math (dropping +0.0 terms).

Engine notes: scores are computed transposed (keys on partitions) with the
two heads' matmuls row-tiled at tile_position (0,0)/(64,0) so they run
concurrently on the PE; exp is one ScalarE activation per key-tile; attn@V
uses a V-augmented ones(keep) column for the softmax denominator.  V is
projected transposed (full-rate matmuls) then flipped per key-tile with PE
transposes; the keep multiply rides on the PSUM->SBUF copy.
"""
import math

import numpy as np

from concourse import bacc, tile, mybir
from concourse.bass_utils import run_bass_kernel_spmd

N_CORES = 8
B, S, D, H = 2, 2048, 1024, 16
DK = D // H                      # 64
HPC = H // N_CORES               # 2 heads per core
R = B * S                        # 4096 query rows
KT = D // 128                    # 8 contraction tiles
NT = D // 128                    # 8 output tiles (phase 3)
QB = 512                         # query block
NQB = S // QB                    # 4 q-blocks per batch
OCW = R // (2 * N_CORES)         # 256 out cols per core per batch

dt = mybir.dt
AF = mybir.ActivationFunctionType
BF16 = dt.bfloat16

_CACHE = {}


def _build(skt, reps=1, variant="", no_collective=False, zb=False):
    """skt = key tiles of 128 per batch after compaction."""
    sk = skt * 128
    bsk = B * sk
    bskt = B * skt

    nc = bacc.Bacc("TRN2", target_bir_lowering=False, debug=False,
                   num_devices=N_CORES)

    W16 = 64 + bsk + 128          # ident stack | keeprow | bk
    W32 = 10 + bskt + 64          # bq | bv | bo | keepcol | ones
    xqT = nc.dram_tensor("xqT", [D, R], BF16, kind="ExternalInput")
    xkT = nc.dram_tensor("xkT", [D, bsk], BF16, kind="ExternalInput")
    xvT = nc.dram_tensor("xvT", [D, bsk], BF16, kind="ExternalInput")
    wqkv = nc.dram_tensor("wqkv", [128, 3 * KT * 128], BF16,
                          kind="ExternalInput")
    wo = nc.dram_tensor("wo", [128, KT * D], BF16, kind="ExternalInput")
    cst16 = nc.dram_tensor("cst16", [128, W16], BF16, kind="ExternalInput")
    cst32 = nc.dram_tensor("cst32", [128, W32], dt.float32,
                           kind="ExternalInput")
    outT = nc.dram_tensor("outT", [D, B * OCW], dt.float32,
                          kind="ExternalOutput")

    f32r = dt.float32r
    rg = [list(range(N_CORES))]

    with tile.TileContext(nc) as tc:
        with tc.tile_pool(name="dram", bufs=1, space="DRAM") as dram:
            for rep in range(reps):
                a2_in = [dram.tile([N_CORES, 128, OCW], BF16,
                                   name=f"a2in{rep}_{b_}") for b_ in range(B)]
                a2_out = [dram.tile([N_CORES, 128, OCW], BF16,
                                    name=f"a2out{rep}_{b_}") for b_ in range(B)]

                pw = tc.alloc_tile_pool(name="pw", bufs=1)
                pqkv = tc.alloc_tile_pool(name="pqkv", bufs=1)

                wq_t = pw.tile([128, KT, 128], BF16, tag="wq")
                wk_t = pw.tile([128, KT, 128], BF16, tag="wk")
                wv_t = pw.tile([128, KT, 128], BF16, tag="wv")
                for wi, t in enumerate((wq_t, wk_t, wv_t)):
                    nc.sync.dma_start(
                        t[:],
                        wqkv[:, wi * KT * 128:(wi + 1) * KT * 128]
                        .rearrange("p (t n) -> p t n", t=KT))
                c16 = pw.tile([128, W16], BF16, tag="c16")
                nc.gpsimd.dma_start(c16[:], cst16[:])
                c32 = pw.tile([128, W32], dt.float32, tag="c32")
                nc.gpsimd.dma_start(c32[:], cst32[:])
                # phase-3 weights: early, on the ACT queue (idle pre-exp)
                wo_t = pw.tile([128, KT, D], BF16, tag="wo")
                nc.scalar.dma_start(
                    wo_t[:], wo[:].rearrange("p (t n) -> p t n", t=KT))

                bq_sb = c32[:, 0:1]
                bv_sb = c32[:, 1:2]
                bk_sb = c16[0:1, 64 + bsk:64 + bsk + 128]
                ones64_t = pw.tile([1, 64], f32r, tag="ones64")
                nc.sync.dma_start(
                    ones64_t[:],
                    cst32[0:1, 10 + bskt:10 + bskt + 64].bitcast(f32r))
                ones64 = ones64_t[:]

                qT_h = pqkv.tile([128, R], BF16, tag="qT")
                kT_h = pqkv.tile([128, bsk], BF16, tag="kT")
                v_aug = pqkv.tile([128, bskt, 130], BF16, tag="vaug")
                oT_sb = pqkv.tile([128, R], BF16, tag="oT")

                # ones(keep) columns of v_aug first (WAW before data cols)
                for h in range(HPC):
                    nc.gpsimd.dma_start(
                        v_aug[:].rearrange("p a (hh f) -> p a hh f",
                                           hh=HPC)[:, :, h, 64:65]
                        .rearrange("p a one -> p (a one)"),
                        cst32[:, 10:10 + bskt])

                pinq = tc.alloc_tile_pool(name="pinq", bufs=2)
                pin = tc.alloc_tile_pool(name="pin", bufs=1)
                psP = tc.alloc_tile_pool(name="psP", bufs=3, space="PSUM")
                psT = tc.alloc_tile_pool(name="psT", bufs=2, space="PSUM")
                if True:
                    # ---- K/V projections (transposed), chunked input loads --
                    vT_sb = pin.tile([128, bsk], BF16, tag="vT")

                    for (w_t, x_dram, dst, isv) in (
                            (wk_t, xkT, kT_h, False),
                            (wv_t, xvT, vT_sb, True)):
                        lstep = 1024 if isv else 512
                        for l0 in range(0, bsk, lstep):
                            lw = min(lstep, bsk - l0)
                            xc = pinq.tile([128, KT, lstep], BF16,
                                           tag=f"xc{isv}",
                                           name=f"xc{isv}_{l0}")
                            # K on the SP queue, V on Pool: transfers
                            # serialize per issuing queue, so spread them
                            (nc.gpsimd if isv else nc.sync).dma_start(
                                xc[:, :, 0:lw],
                                x_dram[:, l0:l0 + lw]
                                .rearrange("(t p) r -> p t r", p=128))
                            for s0 in range(0, lw, 512):
                                c0 = l0 + s0
                                cw = min(512, lw - s0)
                                ps = psP.tile([128, 512], dt.float32, tag="ps")
                                for t in range(KT):
                                    nc.tensor.matmul(
                                        ps[:, 0:cw], w_t[:, t],
                                        xc[:, t, s0:s0 + cw],
                                        start=(t == 0),
                                        stop=((isv or zb) and t == KT - 1))
                                if isv:
                                    nc.vector.tensor_scalar_add(
                                        dst[:, c0:c0 + cw], ps[:, 0:cw],
                                        bv_sb)
                                else:
                                    if not zb:
                                        # keep-masked bias: pad cols stay 0
                                        nc.tensor.matmul(
                                            ps[:, 0:cw], bk_sb,
                                            c16[0:1, 64 + c0:64 + c0 + cw],
                                            start=False, stop=True)
                                    nc.vector.tensor_copy(dst[:, c0:c0 + cw],
                                                          ps[:, 0:cw])

                    # ---- V flip: v_aug[keys, h*65+d] = vT^T * keep ----
                    for kt in range(bskt):
                        for h in range(HPC):
                            pst = psT.tile([128, 64], BF16, tag="pt")
                            nc.tensor.transpose(
                                pst[:],
                                vT_sb[h * 64:(h + 1) * 64,
                                      kt * 128:(kt + 1) * 128],
                                c16[h * 64:(h + 1) * 64, 0:64])
                            nc.vector.tensor_scalar_mul(
                                v_aug[:, kt, h * 65:h * 65 + 64], pst[:],
                                c32[:, 10 + kt:11 + kt])

                    # ---- Q projection, chunked; attention interleaved ----
                    qchunks = []
                    qengs = (nc.scalar, nc.gpsimd, nc.gpsimd, nc.sync)
                    for ci in range(R // 1024):
                        xq_sb = pinq.tile([128, KT, 1024], BF16, tag="xq",
                                          name=f"xq{ci}")
                        qengs[ci].dma_start(
                            xq_sb[:],
                            xqT[:, ci * 1024:(ci + 1) * 1024]
                            .rearrange("(t p) r -> p t r", p=128))
                        if rep > 0 and ci == 0:
                            # cross-rep serializer for reps-differencing
                            nc.gpsimd.dma_start(xq_sb[0:1, 0, 0:1],
                                                outT[0:1, 0:1])
                        qchunks.append(xq_sb)

                    def qproj(ci, pool):
                        xq_sb = qchunks[ci]
                        for s0 in range(2):
                            col = ci * 1024 + s0 * 512
                            ps = pool.tile([128, 512], dt.float32, tag="ps",
                                           name=f"qps{ci}_{s0}")
                            for t in range(KT):
                                nc.tensor.matmul(
                                    ps[:], wq_t[:, t],
                                    xq_sb[:, t, s0 * 512:(s0 + 1) * 512],
                                    start=(t == 0), stop=(t == KT - 1))
                            nc.vector.tensor_scalar_add(
                                qT_h[:, col:col + 512], ps[:], bq_sb)

                    qproj(0, psP)

                    if variant == "p1":
                        for ci in range(1, R // 1024):
                            qproj(ci, psP)

                    psT.release()
                    psP.release()
                    pin.release()

                    aT_sbs = {}

                    def load_aT(b):
                        aT_sb = pw.tile([128, KT, OCW], BF16, tag=f"aT{b}")
                        nc.sync.dma_start(
                            aT_sb[:],
                            a2_out[b][:].rearrange("j p r -> p j r"))
                        aT_sbs[b] = aT_sb

                    def phase3(b, py, pps):
                        aT_sb = aT_sbs[b]
                        yT = py.tile([128, NT, OCW], dt.float32, tag="y",
                                     name=f"yT{b}")
                        for n in range(NT):
                            ps = pps.tile([128, OCW], dt.float32, tag="ps",
                                          name=f"p3ps{b}_{n}")
                            for t in range(KT):
                                nc.tensor.matmul(
                                    ps[:], wo_t[:, t, n * 128:(n + 1) * 128],
                                    aT_sb[:, t],
                                    start=(t == 0), stop=(t == KT - 1))
                            nc.vector.tensor_scalar_add(
                                yT[:, n], ps[:], c32[:, 2 + n:3 + n])
                        nc.sync.dma_start(
                            outT[:, b * OCW:(b + 1) * OCW]
                            .rearrange("(n p) r -> p n r", p=128), yT[:])

                    # ---- attention ----
                    if variant != "p1":
                        CH = 3 if skt % 3 == 0 else 4
                        with (
                            tc.tile_pool(name="p2p", bufs=4) as p2p,
                            tc.tile_pool(name="p2m", bufs=2) as p2m,
                            tc.tile_pool(name="psS", bufs=2, space="PSUM") as psS,
                            tc.tile_pool(name="psO", bufs=2, space="PSUM") as psO,
                            tc.tile_pool(name="psQ", bufs=2, space="PSUM") as psQ,
                            tc.tile_pool(name="p3x", bufs=1) as p3x,
                        ):
                            blk = 0
                            for b in range(B):
                                for q in range(NQB):
                                    blk += 1
                                    if blk % 2 == 0 and blk // 2 < R // 1024:
                                        qproj(blk // 2, psQ)

                                    qcol = b * S + q * QB
                                    po = [psO.tile([65, QB], dt.float32,
                                                   tag="o", name=f"po{h}")
                                          for h in range(HPC)]
                                    for c0 in range(0, skt, CH):
                                        cw = min(CH, skt - c0)
                                        p_chunk = p2p.tile(
                                            [128, CH, 2 * QB], BF16, tag="pch")
                                        for kk in range(c0, c0 + cw):
                                            kt = b * skt + kk
                                            pss = psS.tile([128, 2 * QB],
                                                           dt.float32, tag="s")
                                            for h in range(HPC):
                                                nc.tensor.matmul(
                                                    pss[:, h * QB:(h + 1) * QB],
                                                    kT_h[h * 64:(h + 1) * 64,
                                                         kt * 128:(kt + 1) * 128],
                                                    qT_h[h * 64:(h + 1) * 64,
                                                         qcol:qcol + QB],
                                                    start=True, stop=True,
                                                    tile_position=(h * 64, 0))
                                            nc.scalar.activation(
                                                p_chunk[:, kk - c0], pss[:],
                                                AF.Exp)
                                        for kk in range(c0, c0 + cw):
                                            kt = b * skt + kk
                                            for h in range(HPC):
                                                nc.tensor.matmul(
                                                    po[h][:],
                                                    v_aug[:, kt,
                                                          h * 65:(h + 1) * 65],
                                                    p_chunk[:, kk - c0,
                                                            h * QB:(h + 1) * QB],
                                                    start=(kk == 0),
                                                    stop=(kk == skt - 1))
                                    for h in range(HPC):
                                        rec = p2m.tile([1, QB], f32r,
                                                       tag="rec")
                                        with nc.allow_low_precision(
                                                reason="1/den fp22 is plenty"):
                                            nc.vector.reciprocal(
                                                rec[:], po[h][64:65, :])
                                        pbr = psQ.tile([64, QB], dt.float32,
                                                       tag="ps")
                                        nc.tensor.matmul(pbr[:], ones64,
                                                         rec[:], start=True,
                                                         stop=True)
                                        bc = p2m.tile([64, QB], dt.float32,
                                                      tag="bc")
                                        nc.vector.tensor_copy(bc[:], pbr[:])
                                        nc.vector.tensor_mul(
                                            oT_sb[h * 64:(h + 1) * 64,
                                                  qcol:qcol + QB],
                                            po[h][0:64, :], bc[:])
                                    # stage this block's a2 slice right away
                                    if variant not in ("p12",):
                                        nc.sync.dma_start(
                                            a2_in[b][2 * q:2 * q + 2]
                                            .rearrange("d p r -> p d r"),
                                            oT_sb[:, qcol:qcol + QB]
                                            .rearrange("p (d r) -> p d r", d=2))
                                # per-batch A2A of attention output
                                if variant not in ("p12",):
                                    if no_collective:
                                        nc.sync.dma_start(a2_out[b][:],
                                                          a2_in[b][:])
                                    else:
                                        nc.gpsimd.collective_compute(
                                            "AllToAll", mybir.AluOpType.bypass,
                                            replica_groups=rg,
                                            ins=[a2_in[b].opt()],
                                            outs=[a2_out[b].opt()])
                                    if variant != "p12a":
                                        load_aT(b)
                            # batch-0 out-proj overlaps the batch-1 collective
                            if variant not in ("p12", "p12a"):
                                phase3(0, p3x, psQ)

                    # ---- phase 3 for batch 1 (batch 0 ran inside attention) --
                    if variant not in ("p1", "p12", "p12a"):
                        with (
                            tc.tile_pool(name="p3a", bufs=1) as p3a,
                            tc.tile_pool(name="p3ps", bufs=3,
                                         space="PSUM") as p3ps,
                        ):
                            phase3(1, p3a, p3ps)

                pinq.release()
                pqkv.release()
                pw.release()

    nc.compile()
    return nc


def _prep(query, key, value, mask, Wq, bq, Wk, bk, Wv, bv, Wo, bo):
    bf16 = mybir.dt.np(BF16)
    f = lambda a: np.asarray(a, dtype=np.float32)

    m = np.asarray(mask, dtype=bool).reshape(B, S)
    keep = ~m
    idx = [np.nonzero(keep[b])[0] for b in range(B)]
    nkeep = [len(i) for i in idx]
    skt = max(1, math.ceil(max(nkeep) / 128))
    sk = skt * 128

    key_f = f(key)
    val_f = f(value)
    xk_c = np.zeros((B, sk, D), np.float32)
    xv_c = np.zeros((B, sk, D), np.float32)
    keep01 = np.zeros((B, sk), np.float32)
    for b in range(B):
        xk_c[b, :nkeep[b]] = key_f[b, idx[b]]
        xv_c[b, :nkeep[b]] = val_f[b, idx[b]]
        keep01[b, :nkeep[b]] = 1.0

    xqT = np.ascontiguousarray(f(query).reshape(R, D).T).astype(bf16)
    xkT = np.ascontiguousarray(xk_c.reshape(B * sk, D).T).astype(bf16)
    xvT = np.ascontiguousarray(xv_c.reshape(B * sk, D).T).astype(bf16)

    sdk = np.float32(1.0 / np.sqrt(DK))
    wq_f = f(Wq) * sdk
    bq_f = f(bq) * sdk
    wk_f, wv_f, wo_f = f(Wk), f(Wv), f(Wo)

    # lhsT layout [p, t*128+n] = W[t*128+p, n_slice]
    def wslice(w, c):
        sl = w[:, 128 * c:128 * (c + 1)]           # [D, 128]
        return np.ascontiguousarray(
            sl.reshape(KT, 128, 128).transpose(1, 0, 2).reshape(128, KT * 128)
        ).astype(bf16)

    wo_l = np.ascontiguousarray(
        wo_f.reshape(KT, 128, D).transpose(1, 0, 2).reshape(128, KT * D)
    ).astype(bf16)

    keeprow = keep01.reshape(B * sk)
    keepcol = np.ascontiguousarray(
        keep01.reshape(B, skt, 128).transpose(2, 0, 1).reshape(128, B * skt))
    bskt = B * skt
    bsk = B * sk

    shared = {"xqT": xqT, "xkT": xkT, "xvT": xvT, "wo": wo_l}
    in_maps = []
    for c in range(N_CORES):
        sl = slice(128 * c, 128 * (c + 1))
        c16 = np.zeros((128, 64 + bsk + 128), np.float32)
        c16[:, 0:64] = np.tile(np.eye(64, dtype=np.float32), (2, 1))
        c16[0, 64:64 + bsk] = keeprow
        c16[0, 64 + bsk:] = f(bk)[sl]
        c32 = np.zeros((128, 10 + bskt + 64), np.float32)
        c32[:, 0] = bq_f[sl]
        c32[:, 1] = f(bv)[sl]
        c32[:, 2:10] = f(bo).reshape(NT, 128).T
        c32[:, 10:10 + bskt] = keepcol
        c32[:, 10 + bskt:] = 1.0
        in_maps.append({
            "wqkv": np.ascontiguousarray(np.concatenate(
                [wslice(wq_f, c), wslice(wk_f, c), wslice(wv_f, c)], axis=1)),
            "cst16": c16.astype(bf16),
            "cst32": c32,
            **shared,
        })
    return in_maps, skt


def kernel(query, key, value, mask, Wq, bq, Wk, bk, Wv, bv, Wo, bo):
    in_maps, skt = _prep(query, key, value, mask, Wq, bq, Wk, bk, Wv, bv,
                         Wo, bo)
    zb = not np.asarray(bk, dtype=np.float32).any()
    if _CACHE.get("key") != (skt, zb):
        _CACHE["nc"] = _build(skt, zb=zb)
        _CACHE["key"] = (skt, zb)
    nc = _CACHE["nc"]
    res = run_bass_kernel_spmd(nc, in_maps, list(range(N_CORES)))
    out = np.empty((R, D), np.float32)
    for c in range(N_CORES):
        o = res.results[c]["outT"]                 # [D, B*OCW]
        for b in range(B):
            rows = slice(b * S + c * OCW, b * S + (c + 1) * OCW)
            out[rows] = o[:, b * OCW:(b + 1) * OCW].T
    return out.reshape(B, S, D)


# revision 51
# speedup vs baseline: 7.5399x; 7.5399x over previous
"""Multi-head attention on 8 trn2 NeuronCores — TP-direct + key compaction.

Sharding: pure tensor-parallel for Q/K/V — each core reads the FULL
(bf16, host-transposed) query/key/value activations and projects only its
own 2 heads' 128 output dims (weight slices are tiny), so there are NO
collectives before attention.  The only collective is the AllToAll that
re-shards attention output from head-sharded to row-sharded before the
out-projection, split into two per-batch halves so the first overlaps
batch-1 attention compute.

Key compaction: the mask kills ~half the keys (V rows zeroed + keep-mask
denominator in the reference semantics).  Masked keys contribute exact
zeros, so the host gathers only unmasked key/value rows per batch (padded
to a multiple of 128, keep=0 on pads).  Scores / exp / attn@V shrink by
~2x with bitwise-identical math (dropping +0.0 terms).

Engine notes: scores are computed transposed (keys on partitions) with the
two heads' matmuls row-tiled at tile_position (0,0)/(64,0) so they run
concurrently on the PE; exp is one ScalarE activation per key-tile; attn@V
uses a V-augmented ones(keep) column for the softmax denominator.  V is
projected transposed (full-rate matmuls) then flipped per key-tile with PE
transposes; the keep multiply rides on the PSUM->SBUF copy.
"""
import math

import numpy as np

from concourse import bacc, tile, mybir
from concourse.bass_utils import run_bass_kernel_spmd

N_CORES = 8
B, S, D, H = 2, 2048, 1024, 16
DK = D // H                      # 64
HPC = H // N_CORES               # 2 heads per core
R = B * S                        # 4096 query rows
KT = D // 128                    # 8 contraction tiles
NT = D // 128                    # 8 output tiles (phase 3)
QB = 512                         # query block
NQB = S // QB                    # 4 q-blocks per batch
OCW = R // (2 * N_CORES)         # 256 out cols per core per batch

dt = mybir.dt
AF = mybir.ActivationFunctionType
BF16 = dt.bfloat16

_CACHE = {}


def _build(skt, reps=1, variant="", no_collective=False, zb=False):
    """skt = key tiles of 128 per batch after compaction."""
    sk = skt * 128
    bsk = B * sk
    bskt = B * skt

    nc = bacc.Bacc("TRN2", target_bir_lowering=False, debug=False,
                   num_devices=N_CORES)

    W16 = 64 + bsk + 128          # ident stack | keeprow | bk
    W32 = 10 + bskt + 64          # bq | bv | bo | keepcol | ones
    xqT = nc.dram_tensor("xqT", [D, R], BF16, kind="ExternalInput")
    xkT = nc.dram_tensor("xkT", [D, bsk], BF16, kind="ExternalInput")
    xvT = nc.dram_tensor("xvT", [D, bsk], BF16, kind="ExternalInput")
    wqkv = nc.dram_tensor("wqkv", [128, 3 * KT * 128], BF16,
                          kind="ExternalInput")
    wo = nc.dram_tensor("wo", [128, KT * D], BF16, kind="ExternalInput")
    cst16 = nc.dram_tensor("cst16", [128, W16], BF16, kind="ExternalInput")
    cst32 = nc.dram_tensor("cst32", [128, W32], dt.float32,
                           kind="ExternalInput")
    outT = nc.dram_tensor("outT", [D, B * OCW], dt.float32,
                          kind="ExternalOutput")

    f32r = dt.float32r
    rg = [list(range(N_CORES))]

    with tile.TileContext(nc) as tc:
        with tc.tile_pool(name="dram", bufs=1, space="DRAM") as dram:
            for rep in range(reps):
                a2_in = [dram.tile([N_CORES, 128, OCW], BF16,
                                   name=f"a2in{rep}_{b_}") for b_ in range(B)]
                a2_out = [dram.tile([N_CORES, 128, OCW], BF16,
                                    name=f"a2out{rep}_{b_}") for b_ in range(B)]

                pw = tc.alloc_tile_pool(name="pw", bufs=1)
                pqkv = tc.alloc_tile_pool(name="pqkv", bufs=1)

                wq_t = pw.tile([128, KT, 128], BF16, tag="wq")
                wk_t = pw.tile([128, KT, 128], BF16, tag="wk")
                wv_t = pw.tile([128, KT, 128], BF16, tag="wv")
                for wi, t in ((1, wk_t), (2, wv_t), (0, wq_t)):
                    nc.sync.dma_start(
                        t[:],
                        wqkv[:, wi * KT * 128:(wi + 1) * KT * 128]
                        .rearrange("p (t n) -> p t n", t=KT))
                c16 = pw.tile([128, W16], BF16, tag="c16")
                nc.gpsimd.dma_start(c16[:], cst16[:])
                c32 = pw.tile([128, W32], dt.float32, tag="c32")
                nc.gpsimd.dma_start(c32[:], cst32[:])
                wo_t = pw.tile([128, KT, D], BF16, tag="wo")

                bq_sb = c32[:, 0:1]
                bv_sb = c32[:, 1:2]
                bk_sb = c16[0:1, 64 + bsk:64 + bsk + 128]
                ones64_t = pw.tile([1, 64], f32r, tag="ones64")
                nc.sync.dma_start(
                    ones64_t[:],
                    cst32[0:1, 10 + bskt:10 + bskt + 64].bitcast(f32r))
                ones64 = ones64_t[:]

                qT_h = pqkv.tile([128, R], BF16, tag="qT")
                kT_h = pqkv.tile([128, bsk], BF16, tag="kT")
                v_aug = pqkv.tile([128, bskt, 130], BF16, tag="vaug")
                oT_sb = pqkv.tile([128, R], BF16, tag="oT")

                # ones(keep) columns of v_aug first (WAW before data cols)
                for h in range(HPC):
                    nc.gpsimd.dma_start(
                        v_aug[:].rearrange("p a (hh f) -> p a hh f",
                                           hh=HPC)[:, :, h, 64:65]
                        .rearrange("p a one -> p (a one)"),
                        cst32[:, 10:10 + bskt])

                pinq = tc.alloc_tile_pool(name="pinq", bufs=2)
                pin = tc.alloc_tile_pool(name="pin", bufs=1)
                psP = tc.alloc_tile_pool(name="psP", bufs=3, space="PSUM")
                psT = tc.alloc_tile_pool(name="psT", bufs=2, space="PSUM")
                if True:
                    # ---- K/V projections (transposed), chunked input loads --
                    vT_sb = pin.tile([128, bsk], BF16, tag="vT")

                    for (w_t, x_dram, dst, isv) in (
                            (wk_t, xkT, kT_h, False),
                            (wv_t, xvT, vT_sb, True)):
                        lstep = 1024 if isv else 512
                        for l0 in range(0, bsk, lstep):
                            lw = min(lstep, bsk - l0)
                            xc = pinq.tile([128, KT, lstep], BF16,
                                           tag=f"xc{isv}",
                                           name=f"xc{isv}_{l0}")
                            # V on Pool; K split SP/ACT: transfers
                            # serialize per issuing queue, so spread them
                            (nc.gpsimd if isv else nc.sync).dma_start(
                                xc[:, :, 0:lw],
                                x_dram[:, l0:l0 + lw]
                                .rearrange("(t p) r -> p t r", p=128))
                            for s0 in range(0, lw, 512):
                                c0 = l0 + s0
                                cw = min(512, lw - s0)
                                ps = psP.tile([128, 512], dt.float32, tag="ps")
                                for t in range(KT):
                                    nc.tensor.matmul(
                                        ps[:, 0:cw], w_t[:, t],
                                        xc[:, t, s0:s0 + cw],
                                        start=(t == 0),
                                        stop=((isv or zb) and t == KT - 1))
                                if isv:
                                    nc.vector.tensor_scalar_add(
                                        dst[:, c0:c0 + cw], ps[:, 0:cw],
                                        bv_sb)
                                else:
                                    if not zb:
                                        # keep-masked bias: pad cols stay 0
                                        nc.tensor.matmul(
                                            ps[:, 0:cw], bk_sb,
                                            c16[0:1, 64 + c0:64 + c0 + cw],
                                            start=False, stop=True)
                                    nc.vector.tensor_copy(dst[:, c0:c0 + cw],
                                                          ps[:, 0:cw])

                    # ---- V flip: v_aug[keys, h*65+d] = vT^T * keep ----
                    for kt in range(bskt):
                        for h in range(HPC):
                            pst = psT.tile([128, 64], BF16, tag="pt")
                            nc.tensor.transpose(
                                pst[:],
                                vT_sb[h * 64:(h + 1) * 64,
                                      kt * 128:(kt + 1) * 128],
                                c16[h * 64:(h + 1) * 64, 0:64])
                            nc.vector.tensor_scalar_mul(
                                v_aug[:, kt, h * 65:h * 65 + 64], pst[:],
                                c32[:, 10 + kt:11 + kt])

                    # ---- Q projection, chunked; attention interleaved ----
                    qchunks = []
                    qengs = (nc.scalar, nc.gpsimd, nc.gpsimd, nc.sync)
                    for ci in range(R // 1024):
                        xq_sb = pinq.tile([128, KT, 1024], BF16, tag="xq",
                                          name=f"xq{ci}")
                        qengs[ci].dma_start(
                            xq_sb[:],
                            xqT[:, ci * 1024:(ci + 1) * 1024]
                            .rearrange("(t p) r -> p t r", p=128))
                        if ci == 0:
                            # phase-3 weights ride the ACT queue after xq0
                            nc.scalar.dma_start(
                                wo_t[:],
                                wo[:].rearrange("p (t n) -> p t n", t=KT))
                        if rep > 0 and ci == 0:
                            # cross-rep serializer for reps-differencing
                            nc.gpsimd.dma_start(xq_sb[0:1, 0, 0:1],
                                                outT[0:1, 0:1])
                        qchunks.append(xq_sb)

                    def qproj(ci, pool):
                        xq_sb = qchunks[ci]
                        for s0 in range(2):
                            col = ci * 1024 + s0 * 512
                            ps = pool.tile([128, 512], dt.float32, tag="ps",
                                           name=f"qps{ci}_{s0}")
                            for t in range(KT):
                                nc.tensor.matmul(
                                    ps[:], wq_t[:, t],
                                    xq_sb[:, t, s0 * 512:(s0 + 1) * 512],
                                    start=(t == 0), stop=(t == KT - 1))
                            nc.vector.tensor_scalar_add(
                                qT_h[:, col:col + 512], ps[:], bq_sb)

                    qproj(0, psP)

                    if variant == "p1":
                        for ci in range(1, R // 1024):
                            qproj(ci, psP)

                    psT.release()
                    psP.release()
                    pin.release()

                    aT_sbs = {}

                    def load_aT(b):
                        aT_sb = pw.tile([128, KT, OCW], BF16, tag=f"aT{b}")
                        nc.sync.dma_start(
                            aT_sb[:],
                            a2_out[b][:].rearrange("j p r -> p j r"))
                        aT_sbs[b] = aT_sb

                    def phase3(b, py, pps):
                        aT_sb = aT_sbs[b]
                        yT = py.tile([128, NT, OCW], dt.float32, tag="y",
                                     name=f"yT{b}")
                        for n in range(NT):
                            ps = pps.tile([128, OCW], dt.float32, tag="ps",
                                          name=f"p3ps{b}_{n}")
                            for t in range(KT):
                                nc.tensor.matmul(
                                    ps[:], wo_t[:, t, n * 128:(n + 1) * 128],
                                    aT_sb[:, t],
                                    start=(t == 0), stop=(t == KT - 1))
                            nc.vector.tensor_scalar_add(
                                yT[:, n], ps[:], c32[:, 2 + n:3 + n])
                            nc.sync.dma_start(
                                outT[n * 128:(n + 1) * 128,
                                     b * OCW:(b + 1) * OCW], yT[:, n])

                    # ---- attention ----
                    if variant != "p1":
                        CH = 3 if skt % 3 == 0 else 4
                        with (
                            tc.tile_pool(name="p2p", bufs=4) as p2p,
                            tc.tile_pool(name="p2m", bufs=2) as p2m,
                            tc.tile_pool(name="psS", bufs=2, space="PSUM") as psS,
                            tc.tile_pool(name="psO", bufs=2, space="PSUM") as psO,
                            tc.tile_pool(name="psQ", bufs=2, space="PSUM") as psQ,
                            tc.tile_pool(name="p3x", bufs=1) as p3x,
                        ):
                            blk = 0
                            for b in range(B):
                                for q in range(NQB):
                                    blk += 1
                                    if blk % 2 == 0 and blk // 2 < R // 1024:
                                        qproj(blk // 2, psQ)

                                    qcol = b * S + q * QB
                                    po = [psO.tile([65, QB], dt.float32,
                                                   tag="o", name=f"po{h}")
                                          for h in range(HPC)]
                                    for c0 in range(0, skt, CH):
                                        cw = min(CH, skt - c0)
                                        p_chunk = p2p.tile(
                                            [128, CH, 2 * QB], BF16, tag="pch")
                                        for kk in range(c0, c0 + cw):
                                            kt = b * skt + kk
                                            pss = psS.tile([128, 2 * QB],
                                                           dt.float32, tag="s")
                                            for h in range(HPC):
                                                nc.tensor.matmul(
                                                    pss[:, h * QB:(h + 1) * QB],
                                                    kT_h[h * 64:(h + 1) * 64,
                                                         kt * 128:(kt + 1) * 128],
                                                    qT_h[h * 64:(h + 1) * 64,
                                                         qcol:qcol + QB],
                                                    start=True, stop=True,
                                                    tile_position=(h * 64, 0))
                                            nc.scalar.activation(
                                                p_chunk[:, kk - c0], pss[:],
                                                AF.Exp)
                                        for kk in range(c0, c0 + cw):
                                            kt = b * skt + kk
                                            for h in range(HPC):
                                                nc.tensor.matmul(
                                                    po[h][:],
                                                    v_aug[:, kt,
                                                          h * 65:(h + 1) * 65],
                                                    p_chunk[:, kk - c0,
                                                            h * QB:(h + 1) * QB],
                                                    start=(kk == 0),
                                                    stop=(kk == skt - 1))
                                    for h in range(HPC):
                                        rec = p2m.tile([1, QB], f32r,
                                                       tag="rec")
                                        with nc.allow_low_precision(
                                                reason="1/den fp22 is plenty"):
                                            nc.vector.reciprocal(
                                                rec[:], po[h][64:65, :])
                                        pbr = psQ.tile([64, QB], dt.float32,
                                                       tag="ps")
                                        nc.tensor.matmul(pbr[:], ones64,
                                                         rec[:], start=True,
                                                         stop=True)
                                        bc = p2m.tile([64, QB], dt.float32,
                                                      tag="bc")
                                        nc.vector.tensor_copy(bc[:], pbr[:])
                                        nc.vector.tensor_mul(
                                            oT_sb[h * 64:(h + 1) * 64,
                                                  qcol:qcol + QB],
                                            po[h][0:64, :], bc[:])
                                    # stage this block's a2 slice right away
                                    if variant not in ("p12",):
                                        nc.sync.dma_start(
                                            a2_in[b][2 * q:2 * q + 2]
                                            .rearrange("d p r -> p d r"),
                                            oT_sb[:, qcol:qcol + QB]
                                            .rearrange("p (d r) -> p d r", d=2))
                                # per-batch A2A of attention output
                                if variant not in ("p12",):
                                    if no_collective:
                                        nc.sync.dma_start(a2_out[b][:],
                                                          a2_in[b][:])
                                    else:
                                        nc.gpsimd.collective_compute(
                                            "AllToAll", mybir.AluOpType.bypass,
                                            replica_groups=rg,
                                            ins=[a2_in[b].opt()],
                                            outs=[a2_out[b].opt()])
                                    if variant != "p12a":
                                        load_aT(b)
                            # batch-0 out-proj overlaps the batch-1 collective
                            if variant not in ("p12", "p12a"):
                                phase3(0, p3x, psQ)

                    # ---- phase 3 for batch 1 (batch 0 ran inside attention) --
                    if variant not in ("p1", "p12", "p12a"):
                        with (
                            tc.tile_pool(name="p3a", bufs=1) as p3a,
                            tc.tile_pool(name="p3ps", bufs=3,
                                         space="PSUM") as p3ps,
                        ):
                            phase3(1, p3a, p3ps)

                pinq.release()
                pqkv.release()
                pw.release()

    nc.compile()
    return nc


def _prep(query, key, value, mask, Wq, bq, Wk, bk, Wv, bv, Wo, bo):
    bf16 = mybir.dt.np(BF16)
    f = lambda a: np.asarray(a, dtype=np.float32)

    m = np.asarray(mask, dtype=bool).reshape(B, S)
    keep = ~m
    idx = [np.nonzero(keep[b])[0] for b in range(B)]
    nkeep = [len(i) for i in idx]
    skt = max(1, math.ceil(max(nkeep) / 128))
    sk = skt * 128

    key_f = f(key)
    val_f = f(value)
    xk_c = np.zeros((B, sk, D), np.float32)
    xv_c = np.zeros((B, sk, D), np.float32)
    keep01 = np.zeros((B, sk), np.float32)
    for b in range(B):
        xk_c[b, :nkeep[b]] = key_f[b, idx[b]]
        xv_c[b, :nkeep[b]] = val_f[b, idx[b]]
        keep01[b, :nkeep[b]] = 1.0

    xqT = np.ascontiguousarray(f(query).reshape(R, D).T).astype(bf16)
    xkT = np.ascontiguousarray(xk_c.reshape(B * sk, D).T).astype(bf16)
    xvT = np.ascontiguousarray(xv_c.reshape(B * sk, D).T).astype(bf16)

    sdk = np.float32(1.0 / np.sqrt(DK))
    wq_f = f(Wq) * sdk
    bq_f = f(bq) * sdk
    wk_f, wv_f, wo_f = f(Wk), f(Wv), f(Wo)

    # lhsT layout [p, t*128+n] = W[t*128+p, n_slice]
    def wslice(w, c):
        sl = w[:, 128 * c:128 * (c + 1)]           # [D, 128]
        return np.ascontiguousarray(
            sl.reshape(KT, 128, 128).transpose(1, 0, 2).reshape(128, KT * 128)
        ).astype(bf16)

    wo_l = np.ascontiguousarray(
        wo_f.reshape(KT, 128, D).transpose(1, 0, 2).reshape(128, KT * D)
    ).astype(bf16)

    keeprow = keep01.reshape(B * sk)
    keepcol = np.ascontiguousarray(
        keep01.reshape(B, skt, 128).transpose(2, 0, 1).reshape(128, B * skt))
    bskt = B * skt
    bsk = B * sk

    shared = {"xqT": xqT, "xkT": xkT, "xvT": xvT, "wo": wo_l}
    in_maps = []
    for c in range(N_CORES):
        sl = slice(128 * c, 128 * (c + 1))
        c16 = np.zeros((128, 64 + bsk + 128), np.float32)
        c16[:, 0:64] = np.tile(np.eye(64, dtype=np.float32), (2, 1))
        c16[0, 64:64 + bsk] = keeprow
        c16[0, 64 + bsk:] = f(bk)[sl]
        c32 = np.zeros((128, 10 + bskt + 64), np.float32)
        c32[:, 0] = bq_f[sl]
        c32[:, 1] = f(bv)[sl]
        c32[:, 2:10] = f(bo).reshape(NT, 128).T
        c32[:, 10:10 + bskt] = keepcol
        c32[:, 10 + bskt:] = 1.0
        in_maps.append({
            "wqkv": np.ascontiguousarray(np.concatenate(
                [wslice(wq_f, c), wslice(wk_f, c), wslice(wv_f, c)], axis=1)),
            "cst16": c16.astype(bf16),
            "cst32": c32,
            **shared,
        })
    return in_maps, skt


def kernel(query, key, value, mask, Wq, bq, Wk, bk, Wv, bv, Wo, bo):
    in_maps, skt = _prep(query, key, value, mask, Wq, bq, Wk, bk, Wv, bv,
                         Wo, bo)
    zb = not np.asarray(bk, dtype=np.float32).any()
    if _CACHE.get("key") != (skt, zb):
        _CACHE["nc"] = _build(skt, zb=zb)
        _CACHE["key"] = (skt, zb)
    nc = _CACHE["nc"]
    res = run_bass_kernel_spmd(nc, in_maps, list(range(N_CORES)))
    out = np.empty((R, D), np.float32)
    for c in range(N_CORES):
        o = res.results[c]["outT"]                 # [D, B*OCW]
        for b in range(B):
            rows = slice(b * S + c * OCW, b * S + (c + 1) * OCW)
            out[rows] = o[:, b * OCW:(b + 1) * OCW].T
    return out.reshape(B, S, D)
